# revision 1
# baseline (speedup 1.0000x reference)
"""BatchHardTripletLoss on 8 Trainium2 NeuronCores.

Strategy (row-parallel, per the sharding hint):
  - Host: sort rows by label (the loss is a mean over anchors, so any row
    permutation leaves it unchanged).  After sorting, each anchor's positive
    set is a contiguous candidate range [s, e).
  - Math: with L2-normalized candidates g_ij = f_i_raw . fhat_j and
    r_i = 1/|f_i|:  d2_ij = 2 - 2*r_i*g_ij, and sqrt/affine are monotone:
        hardest_pos = sqrt(max(2 - 2*r*min_pos(g), eps))
        hardest_neg = sqrt(max(2 - 2*r*max_neg(g), eps))
    so only per-row min/max of the gram matrix are needed -- no NxN sqrt,
    and anchors need no normalization (r applied to the scalar extrema).
  - Each core owns 1024 sorted anchors; candidates = all 8192 rows.
  - SPMD: all per-core differences are carried by input *data* (anchor
    slice, window candidate tiles around the core's diagonal block, a
    host-built additive penalty tensor), never by code, so one NEFF runs
    on all 8 cores.
  - Per anchor tile m (128 rows): plain column-tile maxima of the gram
    give hardest-negative candidates; the 2 column tiles holding that
    tile's positives (the "window") are excluded via an additive penalty
    over the 16 partials and replaced by reductions over a separately
    matmul'd window gram ([128,1024]) plus penalty P (P=-100 on [s,e)):
    max(g+P) is the hardest-negative gram, min(g+P)+100 the hardest
    positive one.
  - Engine split: PE matmuls fp32r (full rate); DVE reduces direct from
    PSUM via 3-D APs (2 column tiles per instruction) for 5 of 8 groups;
    ScalarE evicts the other 3 to SBUF where DVE uses 2x-rate
    tensor_scalar+accum; GPSIMD does normalization scaling and the window
    penalty adds; ScalarE computes row norms via Square+accumulate.
"""

import numpy as np

N = 8192
D = 256
NCORES = 8
CA = N // NCORES          # anchors per core
MT = CA // 128            # 8 anchor tiles per core
NT = N // 512             # 16 candidate column tiles
TF = N // 128             # 64 feature row tiles
GRP = 8                   # row tiles per phase-A pipeline group
WMAP = [0, 1, 1, 1, 1, 2, 2, 2]   # local window tile index per anchor tile m
NEVICT = 5                # of the 8 column-tile pairs per m, evicted via ACT
BIG = 100.0
FMIN = float(np.finfo(np.float32).min)

_CACHE = {}


def _build(reps=1):
    import concourse.bass as bass
    import concourse.tile as tile
    from concourse import bacc, mybir, masks
    from contextlib import ExitStack

    F32 = mybir.dt.float32
    F32R = mybir.dt.float32r
    OP = mybir.AluOpType
    AX = mybir.AxisListType
    ACT = mybir.ActivationFunctionType

    nc = bacc.Bacc("TRN2", target_bir_lowering=False, debug=False,
                   num_devices=NCORES)

    feats = nc.dram_tensor("feats", [N, D], F32, kind="ExternalInput").ap()
    anch = nc.dram_tensor("anch", [CA, D], F32, kind="ExternalInput").ap()
    winf = nc.dram_tensor("winf", [2048, D], F32, kind="ExternalInput").ap()
    wm_d = nc.dram_tensor("wm", [128, MT, 1024], F32R, kind="ExternalInput").ap()
    pex_d = nc.dram_tensor("pex", [128, MT, NT], F32, kind="ExternalInput").ap()
    out_d = nc.dram_tensor("out", [1, 1], F32, kind="ExternalOutput").ap()

    with tile.TileContext(nc) as tc:
      for _rep in range(reps):
        with ExitStack() as ctx:
            constp = ctx.enter_context(tc.tile_pool(name="const", bufs=1))
            ident = constp.tile([128, 128], F32)
            masks.make_identity(nc, ident[:])
            ones = constp.tile([128, 1], F32)
            nc.gpsimd.memset(ones[:], 1.0)
            negid = constp.tile([128, 128], F32R)
            nc.scalar.mul(negid[:], ident[:], -BIG)

            vecp = ctx.enter_context(tc.tile_pool(name="vec", bufs=1))
            pex = vecp.tile([128, MT, NT], F32)
            nc.sync.dma_start(pex[:], pex_d[:])
            rna = vecp.tile([128, MT], F32)       # anchor 1/norm

            bigp = ctx.enter_context(tc.tile_pool(name="big", bufs=1))
            fT = bigp.tile([128, 2, N], F32R)     # normalized candidates^T
            wT = bigp.tile([128, 2, 2048], F32R)  # normalized window cands^T
            aT = bigp.tile([128, 2, CA], F32R)    # raw anchors^T

            # ---------------- Phase A: load, normalize, transpose ----------
            with ExitStack() as actx:
                natp = actx.enter_context(tc.tile_pool(name="nat", bufs=3))
                scrp = actx.enter_context(tc.tile_pool(name="ascr", bufs=6))
                sqp = actx.enter_context(tc.tile_pool(name="sq", bufs=3))
                psA = actx.enter_context(
                    tc.tile_pool(name="psA", bufs=4, space="PSUM"))

                def rsqrt_refined(sq, width, tag):
                    """rn ~= 1/sqrt(sq), one Newton step (ACT-sqrt ULP guard)."""
                    y = sqp.tile([128, width], F32, tag=f"{tag}_y")
                    nc.scalar.sqrt(y[:], sq[:])
                    rn = sqp.tile([128, width], F32, tag=f"{tag}_rn")
                    nc.vector.reciprocal(rn[:], y[:])
                    return rn

                def prep_group(src_ap, nat_tile, width, dst, dst_off, tag,
                               scale=True, dve_norm=False):
                    """DMA a [128,width,D] group, normalize rows, transpose
                    into dst[:, k, dst_off:dst_off+width*128]. Returns rn."""
                    nc.sync.dma_start(nat_tile[:], src_ap)
                    sq = sqp.tile([128, width], F32, tag=f"{tag}_sq")
                    if dve_norm:
                        scr = scrp.tile([128, width, D], F32, tag="sq_scrg")
                        nc.vector.tensor_mul(scr[:], nat_tile[:], nat_tile[:])
                        nc.vector.tensor_reduce(
                            sq[:], scr[:], axis=AX.X, op=OP.add)
                    else:
                        for t in range(width):
                            scr = scrp.tile([128, D], F32, tag="sq_scr")
                            nc.scalar.activation(
                                scr[:], nat_tile[:, t, :], ACT.Square,
                                accum_out=sq[:, t:t + 1])
                    rn = rsqrt_refined(sq, width, tag)
                    if scale:
                        for t in range(width):
                            nc.gpsimd.tensor_scalar_mul(
                                nat_tile[:, t, :], nat_tile[:, t, :],
                                rn[:, t:t + 1])
                    for k in range(2):
                        ps = psA.tile([128, 128 * width], F32, tag="pst")
                        for t in range(width):
                            nc.tensor.transpose(
                                ps[:, 128 * t:128 * (t + 1)],
                                nat_tile[:, t, 128 * k:128 * (k + 1)],
                                ident[:])
                        if k == 0:
                            nc.scalar.copy(
                                dst[:, k, dst_off:dst_off + 128 * width],
                                ps[:])
                        else:
                            nc.vector.tensor_copy(
                                dst[:, k, dst_off:dst_off + 128 * width],
                                ps[:])
                    return rn

                natg = natp.tile([128, MT, D], F32, tag="natg")
                rn_a = prep_group(anch.rearrange("(p t) d -> p t d", p=128),
                                  natg, MT, aT, 0, "a", scale=False)
                nc.vector.tensor_copy(rna[:], rn_a[:])
                wv = winf.rearrange("(p t) d -> p t d", p=128)
                for g in range(2):
                    natg = natp.tile([128, GRP, D], F32, tag="natg")
                    prep_group(wv[:, GRP * g:GRP * (g + 1), :], natg,
                               GRP, wT, 128 * GRP * g, f"w{g}")
                featsv = feats.rearrange("(p t) d -> p t d", p=128)
                for g in range(TF // GRP):
                    natg = natp.tile([128, GRP, D], F32, tag="natg")
                    prep_group(featsv[:, GRP * g:GRP * (g + 1), :], natg,
                               GRP, fT, 128 * GRP * g, f"f{g}",
                               dve_norm=(g >= 4))

            # ---------------- Phase B: gram + penalized reductions ---------
            # Group-major: all 8 anchor tiles consume a feature column group
            # right after its prep, so prep and gram work pipeline.
            negall = vecp.tile([128, MT], F32)
            posraw = vecp.tile([128, MT], F32)
            bpall = vecp.tile([128, MT, NT], F32)
            with ExitStack() as bctx:
                psB = bctx.enter_context(
                    tc.tile_pool(name="psB", bufs=3, space="PSUM"))
                psW = bctx.enter_context(
                    tc.tile_pool(name="psW", bufs=1, space="PSUM"))
                sbB = bctx.enter_context(tc.tile_pool(name="sbB", bufs=3))
                bpp = bctx.enter_context(tc.tile_pool(name="bp", bufs=2))
                wmp = bctx.enter_context(tc.tile_pool(name="wmp", bufs=2))

                def window_work(m):
                    lh = [aT[:, k, 128 * m:128 * (m + 1)] for k in range(2)]
                    wMm = wmp.tile([128, 1024], F32R, tag="wMm")
                    nc.sync.dma_start(wMm[:], wm_d[:, m, :])
                    w = WMAP[m]
                    gw = psW.tile([128, 1024], F32, tag="gw")
                    for half in range(2):
                        for k in range(2):
                            nc.tensor.matmul(
                                gw[:, 512 * half:512 * (half + 1)], lh[k],
                                wT[:, k, 512 * (w + half):
                                   512 * (w + half + 1)],
                                start=(k == 0), stop=False)
                        nc.tensor.matmul(
                            gw[:, 512 * half:512 * (half + 1)], negid[:],
                            wMm[:, 512 * half:512 * (half + 1)],
                            start=False, stop=True)
                    gwe = sbB.tile([128, 1024], F32, tag="gwe")
                    nc.scalar.copy(gwe[:], gw[:])
                    winneg = bpp.tile([128, 1], F32, tag="wn")
                    win4 = bpp.tile([128, 4], F32, tag="w4")
                    for half in range(2):
                        sct = sbB.tile([128, 512], F32, tag="sct")
                        nc.vector.tensor_scalar(
                            sct[:], gwe[:, 512 * half:512 * (half + 1)],
                            0.0, None, OP.add, OP.max,
                            accum_out=win4[:, half:half + 1])
                        sct2 = sbB.tile([128, 512], F32, tag="sct2")
                        nc.vector.tensor_scalar(
                            sct2[:], gwe[:, 512 * half:512 * (half + 1)],
                            0.0, None, OP.add, OP.min,
                            accum_out=win4[:, 2 + half:3 + half])
                    nc.vector.tensor_reduce(
                        winneg[:], win4[:, 0:2], axis=AX.X, op=OP.max)
                    nc.vector.tensor_reduce(
                        posraw[:, m:m + 1], win4[:, 2:4], axis=AX.X,
                        op=OP.min)
                    return winneg

                winnegs = [window_work(m) for m in range(MT)]

                def bulk_pair(m, g):
                    lh = [aT[:, k, 128 * m:128 * (m + 1)] for k in range(2)]
                    gps = psB.tile([128, 1024], F32, tag="g")
                    for half in range(2):
                        n = 2 * g + half
                        for k in range(2):
                            nc.tensor.matmul(
                                gps[:, 512 * half:512 * (half + 1)],
                                lh[k],
                                fT[:, k, 512 * n:512 * (n + 1)],
                                start=(k == 0), stop=(k == 1))
                    if (g + m) % 8 < NEVICT:
                        ev = sbB.tile([128, 1024], F32, tag="ev")
                        nc.scalar.copy(ev[:], gps[:])
                        for half in range(2):
                            nc.vector.tensor_scalar(
                                ev[:, 512 * half:512 * (half + 1)],
                                ev[:, 512 * half:512 * (half + 1)],
                                0.0, None, OP.add, OP.max,
                                accum_out=bpall[:, m, 2 * g + half:
                                                2 * g + half + 1])
                    else:
                        nc.vector.tensor_reduce(
                            bpall[:, m, 2 * g:2 * g + 2],
                            gps[:].rearrange("p (a b) -> p a b", a=2),
                            axis=AX.X, op=OP.max)

                for g in range(NT // 2):
                    for m in range(MT):
                        bulk_pair(m, g)

                for m in range(MT):
                    bpx = bpp.tile([128, NT], F32, tag="bpx")
                    nc.vector.tensor_add(
                        bpx[:], bpall[:, m, :], pex[:, m, :])
                    bulkneg = bpp.tile([128, 1], F32, tag="bn")
                    nc.vector.tensor_reduce(
                        bulkneg[:], bpx[:], axis=AX.X, op=OP.max)
                    nc.vector.tensor_tensor(
                        negall[:, m:m + 1], bulkneg[:], winnegs[m][:],
                        op=OP.max)

            # ---------------- Phase C: epilogue ----------------------------
            with ExitStack() as cctx:
                ep = cctx.enter_context(tc.tile_pool(name="ep", bufs=1))
                psC = cctx.enter_context(
                    tc.tile_pool(name="psC", bufs=1, space="PSUM"))

                # hp2 = clip(2 - 2*r*(posraw+BIG)); hn2 = clip(2 - 2*r*negmax)
                hh2 = ep.tile([128, 2, MT], F32)
                nc.vector.tensor_scalar_add(hh2[:, 0, :], posraw[:], BIG)
                nc.vector.tensor_mul(hh2[:, 0, :], hh2[:, 0, :], rna[:])
                nc.vector.tensor_scalar(
                    hh2[:, 0, :], hh2[:, 0, :], -2.0, 2.0, OP.mult, OP.add)
                nc.vector.tensor_mul(hh2[:, 1, :], negall[:], rna[:])
                nc.vector.tensor_scalar(
                    hh2[:, 1, :], hh2[:, 1, :], -2.0, 2.0, OP.mult, OP.add)
                nc.vector.tensor_scalar_max(hh2[:, :, :], hh2[:, :, :], 1e-12)

                hhf = hh2[:].rearrange("p a b -> p (a b)")
                y = ep.tile([128, 2 * MT], F32)
                nc.scalar.sqrt(y[:], hhf)
                # one Newton step for sqrt: y' = 0.5*(y + x/y)
                ry = ep.tile([128, 2 * MT], F32)
                nc.vector.reciprocal(ry[:], y[:])
                nc.vector.tensor_mul(ry[:], ry[:], hhf)
                nc.vector.tensor_add(ry[:], ry[:], y[:])
                nc.vector.tensor_scalar_mul(ry[:], ry[:], 0.5)

                ryv = ry[:].rearrange("p (a b) -> p a b", a=2)
                loss = ep.tile([128, MT], F32)
                nc.vector.tensor_sub(loss[:], ryv[:, 0, :], ryv[:, 1, :])
                nc.vector.tensor_scalar(
                    loss[:], loss[:], 0.3, 0.0, OP.add, OP.max)

                rowsum = ep.tile([128, 1], F32)
                nc.vector.tensor_reduce(
                    rowsum[:], loss[:], axis=AX.X, op=OP.add)
                tot = psC.tile([1, 1], F32)
                nc.tensor.matmul(tot[:], rowsum[:], ones[:],
                                 start=True, stop=True)
                osb = ep.tile([1, 1], F32)
                nc.scalar.copy(osb[:], tot[:])
                nc.sync.dma_start(out_d[:], osb[:])

    nc.compile()
    return nc


def _interleave(x, p=128):
    """[T*p, D] -> row (T*part + t) holds sorted row 128t+part."""
    t = x.shape[0] // p
    return np.ascontiguousarray(
        x.reshape(t, p, x.shape[1]).transpose(1, 0, 2).reshape(t * p, x.shape[1]))


def _prep_inputs(features, labels):
    feats = np.asarray(features, dtype=np.float32)
    labs = np.asarray(labels)
    order = np.argsort(labs, kind="stable")
    sf = np.ascontiguousarray(feats[order])
    sl = labs[order]
    s_g = np.searchsorted(sl, sl, side="left").astype(np.int64)
    e_g = np.searchsorted(sl, sl, side="right").astype(np.int64)

    feats_dev = _interleave(sf)
    jj = np.arange(1024)
    in_maps = []
    for c in range(NCORES):
        rows = slice(CA * c, CA * (c + 1))
        anch_dev = _interleave(sf[rows])
        tg = [(2 * c - 1 + i) % NT for i in range(4)]
        winf_dev = _interleave(
            np.concatenate([sf[512 * t:512 * (t + 1)] for t in tg], axis=0))

        pw = np.zeros((128, MT, 1024), np.float32)
        pex = np.zeros((128, MT, NT), np.float32)
        for m in range(MT):
            t0 = 2 * c - 1 + WMAP[m]
            p0 = CA * c + 128 * m
            s = s_g[p0:p0 + 128] - 512 * t0
            e = e_g[p0:p0 + 128] - 512 * t0
            assert (s >= 0).all() and (e <= 1024).all() and (s < e).all(), \
                f"window containment violated c={c} m={m}"
            pw[:, m, :] = np.where(
                (jj[None, :] >= s[:, None]) & (jj[None, :] < e[:, None]),
                np.float32(1.0), np.float32(0.0))
            for t in (t0, t0 + 1):
                if 0 <= t < NT:
                    pex[:, m, t] = -10.0 * BIG
        in_maps.append({
            "feats": feats_dev, "anch": anch_dev, "winf": winf_dev,
            "wm": pw, "pex": pex,
        })
    return in_maps


def kernel(features, labels):
    from concourse.bass_utils import run_bass_kernel_spmd

    if "nc" not in _CACHE:
        _CACHE["nc"] = _build()
    nc = _CACHE["nc"]

    in_maps = _prep_inputs(features, labels)
    res = run_bass_kernel_spmd(nc, in_maps, core_ids=list(range(NCORES)))
    total = np.float64(0.0)
    for c in range(NCORES):
        total += np.float64(res.results[c]["out"].reshape(())[()])
    return np.float32(total / N)



# revision 6
# speedup vs baseline: 7.2054x; 7.2054x over previous
"""BatchHardTripletLoss on 8 Trainium2 NeuronCores.

Strategy (row-parallel, per the sharding hint):
  - Host: sort rows by label (loss is a mean over anchors -- any permutation
    is loss-invariant), L2-normalize in f32, and hand each core a ROTATED
    copy of the normalized feature matrix, transposed and cast to bf16:
    core c's candidate column j holds sorted row (j + 1024c - 512) mod 8192.
    After rotation every core's 1024 anchors sit at fixed columns
    [512, 1536), and each anchor tile m's positive candidates lie inside a
    fixed 1024-wide window at columns [512*WMAP[m], 512*WMAP[m]+1024).
    All per-core differences are data (rotated ftr, window mask), never
    code, so one NEFF runs SPMD on all 8 cores.
  - Math: with normalized features, d2_ij = 2 - 2*g_ij and sqrt/affine are
    monotone, so only per-row min/max of the gram matrix are needed.
    hardest_neg gram = max over all candidates of (g - 96*pos_mask)
    hardest_pos gram = min over window of (g - 96*pos_mask) + 96
    (the -96 penalty is fused into the PE accumulation as an extra matmul
    of a constant -96*I against the bf16 mask -- no NxN elementwise pass).
  - Device per core: 8 anchor tiles x 8 candidate chunks of 1024; per chunk
    two bf16 matmuls (k-halves of the 256-dim contraction) accumulate the
    gram in PSUM; window chunks get 1-2 extra mask matmuls. PSUM is drained
    by a static split: some chunks reduced directly by DVE (1x from PSUM),
    the rest evicted to SBUF as bf16 by ScalarE and reduced by DVE in 4x
    mode. Tiny f32 epilogue (sqrt via ACT, relu, row-sum, 128-partition sum
    via matmul with ones) produces one scalar per core; host averages.
"""

import numpy as np
import ml_dtypes

N = 8192
D = 256
NCORES = 8
CA = N // NCORES          # anchors per core
MT = CA // 128            # 8 anchor tiles per core
NCH = N // 1024           # 8 candidate chunks of 1024
WMAP = [0, 1, 1, 1, 1, 2, 2, 2]   # window start half (of 512) per anchor tile
BIG = 96.0
AOFF = 512                # anchors occupy rotated cols [512, 1536)

_CACHE = {}


def _build(reps=1):
    import concourse.bass as bass
    import concourse.tile as tile
    from concourse import bacc, mybir, masks
    from contextlib import ExitStack

    F32 = mybir.dt.float32
    BF16 = mybir.dt.bfloat16
    OP = mybir.AluOpType
    AX = mybir.AxisListType

    nc = bacc.Bacc("TRN2", target_bir_lowering=False, debug=False,
                   num_devices=NCORES)

    ftr_d = nc.dram_tensor("ftr", [128, 2, N], BF16, kind="ExternalInput").ap()
    wm_d = nc.dram_tensor("wm", [128, MT, 1024], BF16,
                          kind="ExternalInput").ap()
    out_d = nc.dram_tensor("out", [1, 1], F32, kind="ExternalOutput").ap()

    # Drain policy per (n, m): True -> ACT evict + DVE 4x; False -> DVE 1x.
    # Window chunks (n<2) always evicted (they need max and min), in f32:
    # bf16 would quantize the -96-shifted positives to 0.5 ulp.
    def evicted(n, m):
        return n < 2 or (n * MT + m) % 2 == 0

    with tile.TileContext(nc) as tc:
      for _rep in range(reps):
        with ExitStack() as ctx:
            constp = ctx.enter_context(tc.tile_pool(name="const", bufs=1))
            ident = constp.tile([128, 128], F32)
            masks.make_identity(nc, ident[:])
            ones = constp.tile([128, 1], F32)
            nc.gpsimd.memset(ones[:], 1.0)
            negid = constp.tile([128, 128], BF16)
            nc.scalar.mul(negid[:], ident[:], -BIG)

            bigp = ctx.enter_context(tc.tile_pool(name="big", bufs=1))
            ftr = bigp.tile([128, 2, N], BF16)
            wm = bigp.tile([128, MT, 1024], BF16)

            # DMA: window/anchor chunks first, then the mask, then the rest.
            for n in (0, 1):
                nc.sync.dma_start(ftr[:, :, 1024 * n:1024 * (n + 1)],
                                  ftr_d[:, :, 1024 * n:1024 * (n + 1)])
            nc.sync.dma_start(wm[:], wm_d[:])
            for n in range(2, NCH):
                nc.sync.dma_start(ftr[:, :, 1024 * n:1024 * (n + 1)],
                                  ftr_d[:, :, 1024 * n:1024 * (n + 1)])

            vecp = ctx.enter_context(tc.tile_pool(name="vec", bufs=1))
            bpmax = vecp.tile([128, MT, NCH], F32)
            pmin = vecp.tile([128, MT, 2], F32)

            with ExitStack() as bctx:
                psB = bctx.enter_context(
                    tc.tile_pool(name="psB", bufs=4, space="PSUM"))
                sbB = bctx.enter_context(tc.tile_pool(name="sbB", bufs=3))

                def do_chunk(n, m):
                    lh = [ftr[:, k, AOFF + 128 * m:AOFF + 128 * (m + 1)]
                          for k in range(2)]
                    gps = psB.tile([128, 1024], F32, tag="g")
                    w = WMAP[m]
                    # mask halves (absolute 512-halves w, w+1) in this chunk
                    mh = [h for h in (w, w + 1) if h // 2 == n]
                    for half in range(2):
                        h = 2 * n + half
                        c0 = 512 * half
                        for k in range(2):
                            nc.tensor.matmul(
                                gps[:, c0:c0 + 512], lh[k],
                                ftr[:, k, 512 * h:512 * (h + 1)],
                                start=(k == 0),
                                stop=(k == 1 and h not in mh))
                        if h in mh:
                            nc.tensor.matmul(
                                gps[:, c0:c0 + 512], negid[:],
                                wm[:, m, 512 * (h - w):512 * (h - w) + 512],
                                start=False, stop=True)
                    if evicted(n, m):
                        dt = F32 if mh else BF16
                        ev = sbB.tile([128, 1024], dt, tag=f"ev{dt}")
                        nc.scalar.copy(ev[:], gps[:])
                        dummy = sbB.tile([128, 1024], dt, tag=f"dum{dt}")
                        nc.vector.tensor_scalar(
                            dummy[:], ev[:], 0.0, None, OP.add, OP.max,
                            accum_out=bpmax[:, m, n:n + 1])
                        for h in mh:
                            c0 = 512 * (h % 2)
                            dm2 = sbB.tile([128, 512], dt, tag=f"dm2{dt}")
                            nc.vector.tensor_scalar(
                                dm2[:], ev[:, c0:c0 + 512], 0.0, None,
                                OP.add, OP.min,
                                accum_out=pmin[:, m, h - w:h - w + 1])
                    else:
                        nc.vector.tensor_reduce(
                            bpmax[:, m, n:n + 1], gps[:], axis=AX.X,
                            op=OP.max)
                        for h in mh:
                            c0 = 512 * (h % 2)
                            nc.vector.tensor_reduce(
                                pmin[:, m, h - w:h - w + 1],
                                gps[:, c0:c0 + 512], axis=AX.X, op=OP.min)

                for n in range(NCH):
                    for m in range(MT):
                        do_chunk(n, m)

            # ---------------- epilogue -----------------------------------
            with ExitStack() as cctx:
                ep = cctx.enter_context(tc.tile_pool(name="ep", bufs=1))
                psC = cctx.enter_context(
                    tc.tile_pool(name="psC", bufs=1, space="PSUM"))

                hh2 = ep.tile([128, 2, MT], F32)
                # pos gram = min(pmin) + 96 ; neg gram = max over chunks
                nc.vector.tensor_tensor(
                    hh2[:, 0, :], pmin[:, :, 0], pmin[:, :, 1], op=OP.min)
                nc.vector.tensor_scalar_add(hh2[:, 0, :], hh2[:, 0, :], BIG)
                nc.vector.tensor_reduce(
                    hh2[:, 1, :], bpmax[:], axis=AX.X, op=OP.max)
                # d2 = clip(2 - 2g, eps)
                nc.vector.tensor_scalar(
                    hh2[:, :, :], hh2[:, :, :], -2.0, 2.0, OP.mult, OP.add)
                nc.vector.tensor_scalar_max(hh2[:, :, :], hh2[:, :, :], 1e-12)

                hhf = hh2[:].rearrange("p a b -> p (a b)")
                y = ep.tile([128, 2 * MT], F32)
                nc.scalar.sqrt(y[:], hhf)
                yv = y[:].rearrange("p (a b) -> p a b", a=2)
                loss = ep.tile([128, MT], F32)
                nc.vector.tensor_sub(loss[:], yv[:, 0, :], yv[:, 1, :])
                nc.vector.tensor_scalar(
                    loss[:], loss[:], 0.3, 0.0, OP.add, OP.max)

                rowsum = ep.tile([128, 1], F32)
                nc.vector.tensor_reduce(
                    rowsum[:], loss[:], axis=AX.X, op=OP.add)
                tot = psC.tile([1, 1], F32)
                nc.tensor.matmul(tot[:], rowsum[:], ones[:],
                                 start=True, stop=True)
                osb = ep.tile([1, 1], F32)
                nc.scalar.copy(osb[:], tot[:])
                nc.sync.dma_start(out_d[:], osb[:])

    nc.compile()
    return nc


def _prep_inputs(features, labels):
    feats = np.asarray(features, dtype=np.float32)
    labs = np.asarray(labels)
    order = np.argsort(labs, kind="stable")
    sf = np.ascontiguousarray(feats[order])
    sl = labs[order]
    nrm = np.sqrt((sf.astype(np.float64) ** 2).sum(axis=1, keepdims=True))
    fh = (sf / np.maximum(nrm, 1e-12)).astype(np.float32)
    s_g = np.searchsorted(sl, sl, side="left").astype(np.int64)
    e_g = np.searchsorted(sl, sl, side="right").astype(np.int64)

    jj = np.arange(1024)
    in_maps = []
    for c in range(NCORES):
        off = (CA * c - AOFF) % N
        rot = np.roll(fh, -off, axis=0)          # rot[j] = fh[(j+off) % N]
        ftr = np.ascontiguousarray(
            rot.T.reshape(2, 128, N).transpose(1, 0, 2)
        ).astype(ml_dtypes.bfloat16)

        wm = np.zeros((128, MT, 1024), ml_dtypes.bfloat16)
        for m in range(MT):
            r0 = CA * c + 128 * m                # sorted rows of this tile
            s = (s_g[r0:r0 + 128] - off) % N     # rotated col bounds
            e = (e_g[r0:r0 + 128] - off - 1) % N + 1
            w0 = 512 * WMAP[m]
            assert (s >= w0).all() and (e <= w0 + 1024).all() \
                and (s < e).all(), f"window containment violated c={c} m={m}"
            wm[:, m, :] = np.where(
                (jj[None, :] >= s[:, None] - w0)
                & (jj[None, :] < e[:, None] - w0), 1.0, 0.0)
        in_maps.append({"ftr": ftr, "wm": wm})
    return in_maps


def kernel(features, labels):
    from concourse.bass_utils import run_bass_kernel_spmd

    if "nc" not in _CACHE:
        _CACHE["nc"] = _build()
    nc = _CACHE["nc"]

    in_maps = _prep_inputs(features, labels)
    res = run_bass_kernel_spmd(nc, in_maps, core_ids=list(range(NCORES)))
    total = np.float64(0.0)
    for c in range(NCORES):
        total += np.float64(res.results[c]["out"].reshape(())[()])
    return np.float32(total / N)
